# revision 69
# baseline (speedup 1.0000x reference)
"""Trainium2 Bass kernel for HF-style causal self-attention (B=2, S=2048, D=2048,
H=16, head_dim=128), tensor-parallel over heads across 8 NeuronCores.

Sharding: core c computes heads {2c, 2c+1} for both batches (column-sharded
Wq/Wk/Wv). After per-head attention, an 8-rank AllToAll redistributes the
per-head context from head-sharding to token-sharding, and each core runs the
output projection (full Wo) for its 512-token slice. The host concatenates
the 8 token slices.

Matmul operands are fp16 (1 cycle/row on PE, FWL weight loads), EXCEPT the
Q/K/V projections of token groups 1-3 and 5-7, which run fp8-e4m3 with
perf_mode=DoubleRow: operands are pair-tiled [128, 2, n] so each matmul
contracts 256 deep in ~n cycles -- 1.77x the fp16 projection rate. Groups 0
and 4 (each batch's first 512 tokens) stay fp16: early tokens attend to few
keys, so their q/k/v quantization errors pass the softmax unattenuated,
while late tokens average ~1000 keys and fp8 noise washes out (measured
max-rel 4.2e-3 vs the 2e-2 gate; fp16 baseline was 4.4e-4). Weights are
pre-scaled x32 before e4m3 quantization (else half the N(0, 1/D) weights
land subnormal) and the 1/32 is folded into the PSUM evacuations.
Attention and o_proj stay fp16 (attention contracts only 128 so DoubleRow
can't pair; o_proj feeds the output directly and fp8 there fails the
early-token error budget). PSUM accumulation is fp32; the output is fp32.

Schedule (the point of this version): a soft pipeline. Projections run as
separate Q/K/V SWEEPS (one accumulator pair at a time, so projections +
attention PSUM pools coexist inside 8 banks), and the 32 attention units are
hand-placed into slots between sweeps so the softmax EXPs (ACT engine, the
attention bottleneck at ~110us total) hide under projection matmuls:

  A: per batch-0 group: Q/K sweeps + RoPE + V sweeps; V-only for batch-1
     groups; early batch-0 units interleave after their deps land.
  B: per batch-1 group: Q/K sweeps + RoPE, then the (b0 leftovers and) b1
     units of the matching group. The h0 AllToAll fires before the last h1
     unit; the h1 AllToAll right after it.
  C: o_proj pass 1 (head-0 features, under the h1 collective), pass 2.

Attention unit math: scores^T [k,q] chunks on PE, causally trimmed via a
runtime mask classification; exp on ACT (scale 1/sqrt(hd), no
max-subtraction -- unit-variance inputs); causal mask applied POST-exp as a
0/1 fp16 multiply; softmax denominators by DVE elementwise accumulation of
prob chunks + ONE ones-matmul per unit; normalization fused into the PSUM
evacuation (reciprocal+multiply on DVE).
"""

import math
import os

import numpy as np

# ---------------------------------------------------------------- constants
B, S, D = 2, 2048, 2048
H, HD = 16, 128
N_CORES = 8
LOCAL_H = H // N_CORES  # 2 heads per core
LOCAL_F = LOCAL_H * HD  # 256 local features
TOKS = B * S  # 4096 flat tokens (batch-major)
TG = 512  # token-group width (matmul moving dim)
NT = TOKS // TG  # 8 token groups
NB = S // 128  # 16 key blocks per batch
QB = TG // 128  # 4 query blocks per group
ROPE_BASE = 10000.0
SCALE = 1.0 / math.sqrt(HD)
SKIP_THRESH = -1e8  # mask block entirely masked if all values below this
LA = 4  # scores/exp chunks emitted ahead of PV (hides ACT latency from PE)

# fp8 (e4m3, DoubleRow) config: projections for all token groups EXCEPT the
# first 512 tokens of each batch (groups 0 and 4) run with fp8 weights+x --
# early tokens attend to few keys, so their q/k/v errors pass through the
# softmax unattenuated; late tokens average ~1000 keys and fp8 noise washes
# out. o_proj runs fp8 for each core's token sub-blocks 1-3 (sub-block 0
# holds the global early tokens on cores 0 and 4). Measured (numpy emulation
# vs fp32 reference): max-rel 5.9e-3 vs the 2e-2 gate.
FP8_GROUPS = (1, 2, 3, 5, 6, 7)
WSCALE = 32.0   # weight pre-scale before e4m3 quantization
AF8S = 8.0      # attention-out pre-scale for the fp8 o_proj path
OSCALE = WSCALE * AF8S  # o_proj PSUM carries OSCALE * value
ND8 = D // 256  # 8 paired contraction chunks for DoubleRow
# o_proj feature-pair order: pass-1 pairs (even heads), then pass-2 (odd)
O_PAIRS = [(0, 2), (4, 6), (8, 10), (12, 14), (1, 3), (5, 7), (9, 11),
           (13, 15)]

_NC_CACHE: dict = {}
last_exec_time_ns = None


# ---------------------------------------------------------------- host prep
def _rope_tables():
    inv_freq = 1.0 / (ROPE_BASE ** (np.arange(0, HD, 2, dtype=np.float64) / HD))
    t = np.arange(S, dtype=np.float64)
    freqs = np.outer(t, inv_freq)  # [S, HD/2]
    emb = np.concatenate([freqs, freqs], axis=-1)  # [S, HD]
    cos = np.cos(emb).T  # [HD, S]
    sin = np.sin(emb).T
    # rotate_half with the sign folded into a partition-aligned sin table:
    #   t2[0:64]   = s[64:128] * sinC[64:128]   (sinC[64:] = -sin[0:64])
    #   t2[64:128] = s[0:64]   * sinC[0:64]     (sinC[:64] = sin[64:128])
    # (DVE requires both SBUF inputs at the same base partition, so the
    # table rows live at the SOURCE partition of s, not the output's)
    half = HD // 2
    sinc = np.empty_like(sin)
    sinc[:half] = sin[half:]
    sinc[half:] = -sin[:half]
    return (np.ascontiguousarray(cos.astype(np.float16)),
            np.ascontiguousarray(sinc.astype(np.float16)))


def _classify_mask(mask2d):
    """Per 128x128 block of mask[q, k]: 0=all-zero, 1=needs mul, 2=fully masked."""
    nq, nk = S // 128, S // 128
    blocks = mask2d.reshape(nq, 128, nk, 128)
    mx = blocks.max(axis=(1, 3))
    mn = blocks.min(axis=(1, 3))
    cls = np.ones((nq, nk), dtype=np.int8)
    cls[(mx == 0.0) & (mn == 0.0)] = 0
    cls[mx < SKIP_THRESH] = 2
    return cls


def _build_plan(cls):
    """For each (query group g, key block j): None if skipped, else
    (comp_start_lqb, mul_start_lqb, mul_nblocks). The mul range spans the
    first to last local query block needing the 0/1 mask multiply."""
    plan = {}
    for g in range(4):
        for j in range(NB):
            lcls = [cls[4 * g + l, j] for l in range(QB)]
            if all(c == 2 for c in lcls):
                continue
            comp = min(l for l in range(QB) if lcls[l] != 2)
            muls = [l for l in range(comp, QB) if lcls[l] != 0]
            if muls:
                plan[(g, j)] = (comp, muls[0], muls[-1] - muls[0] + 1)
            else:
                plan[(g, j)] = (comp, 0, 0)
    return plan


def _plan_key(plan):
    return tuple(sorted((k, v) for k, v in plan.items()))


# ---------------------------------------------------------------- bass build
def _build(plan, n_mask_blocks, mask_idx):
    import concourse.bacc as bacc
    import concourse.mybir as mybir
    import concourse.tile as tile

    f32 = mybir.dt.float32
    mm = mybir.dt.float16
    f8 = mybir.dt.float8e4

    nc = bacc.Bacc("TRN2", target_bir_lowering=False, debug=False,
                   num_devices=N_CORES)

    # fp16 x, host-pretiled per (group in {0,4}, j): each [128, TG] tile is
    # one contiguous 128KB DRAM block (the old xT [D, TOKS] view produced
    # 1KB-strided descriptor chains that throttled the t=0 stream)
    xT = nc.dram_tensor("xT", [2, D // 128, 128, TG], mm,
                        kind="ExternalInput").ap()
    wq = nc.dram_tensor("wq", [D, LOCAL_F], mm, kind="ExternalInput").ap()
    wk = nc.dram_tensor("wk", [D, LOCAL_F], mm, kind="ExternalInput").ap()
    wv = nc.dram_tensor("wv", [D, LOCAL_F], mm, kind="ExternalInput").ap()
    # fp8 pair-tiled operands (DoubleRow): x8[gi][j8] pairs d-chunks
    # (256j8+128i+p); w*8[j8] likewise; wo8[ni][pi] pairs feature chunks
    x8d = nc.dram_tensor("x8", [len(FP8_GROUPS), ND8, 128, 2, TG], f8,
                         kind="ExternalInput").ap()
    wq8d = nc.dram_tensor("wq8", [ND8, 128, 2, LOCAL_F], f8,
                          kind="ExternalInput").ap()
    wk8d = nc.dram_tensor("wk8", [ND8, 128, 2, LOCAL_F], f8,
                          kind="ExternalInput").ap()
    wv8d = nc.dram_tensor("wv8", [ND8, 128, 2, LOCAL_F], f8,
                          kind="ExternalInput").ap()
    wo = nc.dram_tensor("wo", [D, D], mm, kind="ExternalInput").ap()
    maskc = nc.dram_tensor("maskc", [max(n_mask_blocks, 1), 128, 512], mm,
                           kind="ExternalInput").ap()
    cosT = nc.dram_tensor("cosT", [HD, S], mm, kind="ExternalInput").ap()
    sinT = nc.dram_tensor("sinT", [HD, S], mm, kind="ExternalInput").ap()
    onesd = nc.dram_tensor("onesd", [128, 128], mm, kind="ExternalInput").ap()
    out = nc.dram_tensor("out", [TG, D], f32, kind="ExternalOutput").ap()

    with tile.TileContext(nc) as tc:
        with (
            tc.tile_pool(name="const", bufs=1) as constp,
            tc.tile_pool(name="dram", bufs=1, space="DRAM") as dram,
        ):
            cos_t = constp.tile([HD, S], mm, tag="cos")
            sin_t = constp.tile([HD, S], mm, tag="sin")
            ones_t = constp.tile([128, 128], mm, tag="ones")

            _attention_body(nc, tc, tile, mybir, mm, plan, mask_idx,
                            cos_t, sin_t, ones_t,
                            xT, wq, wk, wv, wo, out, dram,
                            maskc, cosT, sinT, onesd,
                            x8d, wq8d, wk8d, wv8d)

    nc.compile()
    return nc


def _attention_body(nc, tc, tile, mybir, mm, plan, mask_idx,
                    cos_t, sin_t, ones_t,
                    xT, wq, wk, wv, wo, out, dram,
                    maskc, cosT, sinT, onesd,
                    x8d, wq8d, wk8d, wv8d):
    f32 = mybir.dt.float32
    f8 = mybir.dt.float8e4
    Exp = mybir.ActivationFunctionType.Exp
    DR = mybir.MatmulPerfMode.DoubleRow
    ND = D // 128  # 16 contraction chunks
    half = HD // 2

    inb = [dram.tile([N_CORES, HD, TG], mm, name=f"inb{i}")
           for i in range(LOCAL_H)]
    outb = [dram.tile([N_CORES, HD, TG], mm, name=f"outb{i}")
            for i in range(LOCAL_H)]

    # pools that must survive into phase C (qt doubles as the o_proj
    # pass-1 stash; wop holds the Wo tiles)
    stack = [
        tc.tile_pool(name="qkres", bufs=LOCAL_H),
        tc.tile_pool(name="wop", bufs=36),
    ]
    (qkres, wop) = [s.__enter__() for s in stack]
    # attention-era pools: closed before phase C so afull/outp fit in SBUF
    stack_attn = [
        tc.tile_pool(name="vres", bufs=LOCAL_H * 4 * NT),
        tc.tile_pool(name="wpool", bufs=3 * ND),
        tc.tile_pool(name="wpool8", bufs=3 * ND8),
        tc.tile_pool(name="xpool", bufs=16),
        tc.tile_pool(name="xpool8", bufs=10),
        tc.tile_pool(name="xpoolB", bufs=16),
        tc.tile_pool(name="ropes", bufs=3),
        tc.tile_pool(name="ropet", bufs=2),
        tc.tile_pool(name="ropeu", bufs=2),
        tc.tile_pool(name="maskp", bufs=1),
        tc.tile_pool(name="probs", bufs=10),
        tc.tile_pool(name="saccp", bufs=4),
        tc.tile_pool(name="recipp", bufs=3),
        tc.tile_pool(name="attnp", bufs=6),
    ]
    (vres, wpool, wpool8, xpool, xpool8, xpoolB, ropes, ropet1, ropet2,
     maskp, probs, saccp, recipp, attnp) = [s.__enter__() for s in stack_attn]

    # resident Q^T / K^T per local head [128, TOKS] (fp16)
    qt = [qkres.tile([HD, TOKS], mm, tag="qt", name=f"qt{i}") for i in range(LOCAL_H)]
    kt = [qkres.tile([HD, TOKS], mm, tag="kt", name=f"kt{i}") for i in range(LOCAL_H)]
    # resident V tiles [128 tok, HD] per (local head, flat token block):
    # per-head contiguous so the PV matmul's LDWEIGHTS is FWL-eligible
    v_sb = [[vres.tile([128, HD], mm, tag="v", name=f"v{h}_{i}")
             for i in range(TOKS // 128)] for h in range(LOCAL_H)]

    wq_t = [wpool.tile([128, LOCAL_F], mm, tag="w", name=f"wqt{i}") for i in range(ND)]
    wk_t = [wpool.tile([128, LOCAL_F], mm, tag="w", name=f"wkt{i}") for i in range(ND)]
    wv_t = [wpool.tile([128, LOCAL_F], mm, tag="w", name=f"wvt{i}") for i in range(ND)]
    wq8_t = [wpool8.tile([128, 2, LOCAL_F], f8, tag="w8", name=f"wq8t{i}")
             for i in range(ND8)]
    wk8_t = [wpool8.tile([128, 2, LOCAL_F], f8, tag="w8", name=f"wk8t{i}")
             for i in range(ND8)]
    wv8_t = [wpool8.tile([128, 2, LOCAL_F], f8, tag="w8", name=f"wv8t{i}")
             for i in range(ND8)]

    # constants + mask strips on the scalar DMA queue: off the sync queue
    # (x/weight streaming) and off gpsimd (wo prefetch + collectives)
    nc.scalar.dma_start(out=cos_t, in_=cosT)
    nc.scalar.dma_start(out=sin_t, in_=sinT)
    nc.scalar.dma_start(out=ones_t, in_=onesd)

    # Q/K weights (fp16 + fp8) ride the gpsimd queue BEFORE the barrier
    # trigger: that queue is otherwise idle until the barrier, and this
    # sheds 3MB from the sync/scalar queues during the t=0 ramp, when the
    # j-loop consumes 320KB every ~1.6us and any queue lag stalls the PE.
    # The barrier trigger moves ~15us later, still finishing long before
    # its dependents (mask strips, needed at the first unit ~190us).
    # Interleaved wq/wk order matches the j-loop's consumption order.
    for j in range(ND):
        nc.gpsimd.dma_start(out=wq_t[j], in_=wq[128 * j:128 * (j + 1), :])
        nc.gpsimd.dma_start(out=wk_t[j], in_=wk[128 * j:128 * (j + 1), :])
        nc.gpsimd.dma_start(out=wv_t[j], in_=wv[128 * j:128 * (j + 1), :])
    for j8 in range(ND8):
        nc.gpsimd.dma_start(out=wq8_t[j8], in_=wq8d[j8])
        nc.gpsimd.dma_start(out=wk8_t[j8], in_=wk8d[j8])
        nc.gpsimd.dma_start(out=wv8_t[j8], in_=wv8d[j8])

    # mask strips ride the gpsimd queue BEFORE the barrier: deduped by
    # content on the host (a causal mask has ONE distinct 128KB strip), so
    # they cost nothing at t=0 and are resident for the early attention
    # units that now run inside phase A.
    mask_tiles = {}
    mt_of_idx = {}
    for key, (idx, nb) in sorted(mask_idx.items()):
        if idx not in mt_of_idx:
            mt = maskp.tile([128, 512], mm, tag=f"mb{idx}", name=f"mb{idx}")
            nc.gpsimd.dma_start(out=mt, in_=maskc[idx])
            mt_of_idx[idx] = mt
        mask_tiles[key] = mt_of_idx[idx]

    # a tiny rendezvous at kernel start aligns the cores while phase A's
    # DMAs stream, so the real collectives later pay less skew
    barrier_in = dram.tile([N_CORES, 128, 2], mm, name="barrier_in")
    barrier_out = dram.tile([N_CORES, 128, 2], mm, name="barrier_out")
    nc.gpsimd.collective_compute(
        "AllToAll", mybir.AluOpType.bypass,
        replica_groups=[list(range(N_CORES))],
        ins=[barrier_in.opt()], outs=[barrier_out.opt()],
    )

    wo_t = {}
    # Load order = consumption order: evens (o_proj pass 1, head-0 features)
    # for all n-groups, then odds (pass 2). One shared tag, so slots released
    # by pass 1 recycle into odd-tile prefetch while pass 1 still runs.
    _wo_order = ([(n, f) for n in range(4) for f in range(0, ND, 2)]
                 + [(n, f) for n in range(4) for f in range(1, ND, 2)])

    def load_wo(k):
        n, f = _wo_order[k]
        w_t = wop.tile([128, TG], mm, tag="wo", name=f"wo{n}_{f}")
        nc.gpsimd.dma_start(out=w_t,
                            in_=wo[128 * f:128 * (f + 1),
                                   TG * n:TG * (n + 1)])
        wo_t[(n, f)] = w_t

    # The first 36 wo tiles (all of pass 1 + 4 odds) are emitted up front
    # on the gpsimd queue but BEHIND a gate DMA whose input is written by
    # phase B's sweep t=4 RoPE -- so Wo streams during phase B's DMA lull
    # instead of competing with x/weights at t=0 or with the af/out traffic
    # in phase C. The remaining 28 are emitted inside the o_proj loop AFTER
    # the collective triggers, so the in-order gpsimd queue can't cycle
    # (they wait on pass-1 slot releases, which need AllToAll-1).
    _wo_next = [36]

    def emit_gated_wo_loads(gate_src):
        gate_dram = dram.tile([128, 4], mm, name="wo_gate")
        nc.gpsimd.dma_start(out=gate_dram, in_=gate_src)
        for k in range(36):
            load_wo(k)

    def load_wo_upto(k):
        while _wo_next[0] <= k:
            load_wo(_wo_next[0])
            _wo_next[0] += 1

    def rope_evac(ps, dst, csl, evac_eng, scale=1.0):
        # RoPE on DVE: dst = s*cos + rot_half(s)*sin_signed
        # scale un-does the fp8 weight pre-scale during the PSUM evacuation
        s_t = ropes.tile([HD, TG], mm, tag="s")
        evac_eng(s_t, ps, scale)  # fp32 PSUM -> fp16 SBUF
        t1 = ropet1.tile([HD, TG], mm, tag="t1")
        nc.vector.tensor_mul(t1, s_t, cos_t[:, csl])
        t2 = ropet2.tile([HD, TG], mm, tag="t2")
        nc.vector.tensor_mul(t2[:half], s_t[half:], sin_t[half:, csl])
        nc.vector.tensor_mul(t2[half:], s_t[:half], sin_t[:half, csl])
        nc.vector.tensor_add(dst, t1, t2)

    def scalar_evac(dst, src, scale=1.0):
        if scale == 1.0:
            nc.scalar.copy(dst, src)
        else:
            nc.scalar.mul(dst, src, scale)

    def vector_evac(dst, src, scale=1.0):
        if scale == 1.0:
            nc.vector.tensor_copy(dst, src)
        else:
            nc.vector.tensor_scalar_mul(dst, src, scale)

    def qk_sweeps(t, xdma, evac_eng, xts_pre=None):
        """Q then K projection sweep for token group t + RoPE evacuation.
        Returns nothing; qt/kt columns for group t become valid.
        xts_pre: x tiles already resident in SBUF (phase A t=4 leftovers)."""
        tsl = slice(TG * t, TG * (t + 1))
        csl = slice(TG * (t % 4), TG * (t % 4 + 1))
        fp8g = t in FP8_GROUPS
        if fp8g:
            gi = FP8_GROUPS.index(t)
            xts = []
            for wi, (w8t, res) in enumerate(((wq8_t, qt), (wk8_t, kt))):
                acc = [psqk.tile([HD, TG], f32, tag="qk", name=f"qk{h}")
                       for h in range(LOCAL_H)]
                for j8 in range(ND8):
                    if wi == 0:
                        x_t = xpool8.tile([128, 2, TG], f8, tag="x8")
                        xdma(out=x_t, in_=x8d[gi][j8])
                        xts.append(x_t)
                    for h in range(LOCAL_H):
                        hsl = slice(128 * h, 128 * (h + 1))
                        nc.tensor.matmul(acc[h], w8t[j8][:, :, hsl], xts[j8],
                                         start=(j8 == 0), stop=(j8 == ND8 - 1),
                                         perf_mode=DR)
                for h in range(LOCAL_H):
                    rope_evac(acc[h], res[h][:, tsl], csl, evac_eng,
                              1.0 / WSCALE)
            return xts
        xts = [] if xts_pre is None else xts_pre
        for wi, (w_t, wext, res) in enumerate(((wq_t, wq, qt),
                                               (wk_t, wk, kt))):
            acc = [psqk.tile([HD, TG], f32, tag="qk", name=f"qk{h}")
                   for h in range(LOCAL_H)]
            for j in range(ND):
                if wi == 0 and xts_pre is None:
                    x_t = xpoolB.tile([128, TG], mm, tag="x")
                    xdma(out=x_t, in_=xT[t // 4][j])
                    xts.append(x_t)
                for h in range(LOCAL_H):
                    hsl = slice(128 * h, 128 * (h + 1))
                    nc.tensor.matmul(acc[h], w_t[j][:, hsl], xts[j],
                                     start=(j == 0), stop=(j == ND - 1))
            for h in range(LOCAL_H):
                rope_evac(acc[h], res[h][:, tsl], csl, evac_eng)
        return xts

    PS = {}  # active PSUM pools for the attention units (phase A vs B)

    def emit_scores(h, b, g, j, pe):
        comp, a0, nb = pe
        co = 128 * comp
        qsl = slice(2048 * b + TG * g, 2048 * b + TG * (g + 1))
        ksl = slice(2048 * b + 128 * j, 2048 * b + 128 * (j + 1))
        sc = PS['sc'].tile([128, TG], f32, tag="sc", name="sc")
        nc.tensor.matmul(sc[:, co:], kt[h][:, ksl],
                         qt[h][:, qsl][:, co:],
                         start=True, stop=True)
        pt = probs.tile([128, TG], mm, tag="p", name="pt")
        nc.scalar.activation(pt[:, co:], sc[:, co:], Exp, scale=SCALE)
        if nb:
            # causal mask applied POST-exp as a 0/1 fp16 multiply (all-SBUF
            # 2-byte DVE op); raw scores are O(5) so exp never overflows
            mt = mask_tiles[(g, j)]
            q0 = 128 * a0
            nc.vector.tensor_mul(pt[:, q0:q0 + 128 * nb],
                                 pt[:, q0:q0 + 128 * nb], mt[:, :128 * nb])
        return pt

    def emit_unit(h, b, g):
        """One attention unit: all key chunks of (head h, batch b, query
        group g), pipelined LA chunks deep, ending with the denominator
        reduce + normalize + bounce-buffer write."""
        chunks = [(j, plan[(g, j)]) for j in range(NB) if (g, j) in plan]
        n = len(chunks)
        pv_ps = PS['pv'].tile([HD, TG], f32, tag="pv", name="pv")
        sa = saccp.tile([128, TG], mm, tag="sa", name="sacc")
        pts = []
        for i in range(n + LA):
            if i < n:
                j, pe = chunks[i]
                pts.append(emit_scores(h, b, g, j, pe))
            k = i - LA
            if k < 0 or k >= n:
                continue
            j, (comp, a0, nb) = chunks[k]
            pt = pts[k]
            co = 128 * comp
            first, last = k == 0, k == n - 1
            # softmax denominator partials: elementwise accumulate the prob
            # chunk on DVE (fp16 all-SBUF, 2x; gpsimd's software tensor_add
            # measured slower); partition-reduced once per unit by a single
            # ones-matmul below
            if first:
                nc.vector.tensor_copy(sa, pt)
            else:
                nc.vector.tensor_add(sa[:, co:], sa[:, co:], pt[:, co:])
            kb = 16 * b + j  # flat token block of this key chunk
            nc.tensor.matmul(pv_ps[:, co:], v_sb[h][kb], pt[:, co:],
                             start=first, stop=last)
        sum_ps = PS['sc'].tile([128, TG], f32, tag="sc", name="sum")
        nc.tensor.matmul(sum_ps, ones_t, sa, start=True, stop=True)
        rec = recipp.tile([128, TG], f32, tag="rec")
        # ~18-bit reciprocal; sums are in [1, ~5e3] so no edge cases
        nc.vector.reciprocal_approx_fast(out=rec, in_=sum_ps)
        at = attnp.tile([HD, TG], mm, tag="at")
        nc.vector.tensor_mul(at, pv_ps, rec)
        nc.sync.dma_start(out=inb[h][4 * b + g], in_=at)

    # ---------------- phase A: b0 full QKV + b1 V, interleaved j-loop
    # (long Q/K matmuls + rotating PSUM banks keep every LDWEIGHTS hidden;
    # consecutive matmuls must not accumulate into the SAME bank -- the
    # fill cannot overlap the drain and each MM pays ~40ns)
    x4_tiles = []
    # Early-unit placement: phase A's t>=5 stretch runs V-only sweeps (4
    # PSUM banks) with an idle ACT engine, while phase B's tail is
    # ACT-drain-bound (every unit carries ~1.4x more exp time than PE
    # time). Moving the four smallest b0 units here widens the exp window
    # by ~40us and pulls the whole AllToAll-2 trigger chain earlier.
    A_UNITS = {5: [(0, 0, 0), (1, 0, 0)], 6: [(0, 0, 1), (0, 0, 2)],
               7: [(1, 0, 1)]}
    with tc.tile_pool(name="psA", bufs=8, space="PSUM") as psA:
        for t in range(5):
            b0 = t < 4
            fp8g = t in FP8_GROUPS
            tsl = slice(TG * t, TG * (t + 1))
            csl = slice(TG * (t % 4), TG * (t % 4 + 1))
            if b0:
                acc = [psA.tile([HD, TG], f32, tag="qk", name=f"acc{i}")
                       for i in range(2 * LOCAL_H)]
            vacc = [psA.tile([128, TG], f32, tag="qk", name=f"vacc{i}")
                    for i in range(4)]
            if fp8g:
                # DoubleRow fp8: 8 paired 256-deep contraction chunks
                gi = FP8_GROUPS.index(t)
                for j8 in range(ND8):
                    x_t = xpool8.tile([128, 2, TG], f8, tag="x8")
                    (nc.sync if j8 % 2 == 0 else nc.scalar).dma_start(
                        out=x_t, in_=x8d[gi][j8])
                    if b0:
                        for wi, w8t in enumerate((wq8_t, wk8_t)):
                            for h in range(LOCAL_H):
                                hsl = slice(128 * h, 128 * (h + 1))
                                nc.tensor.matmul(
                                    acc[2 * wi + h], w8t[j8][:, :, hsl], x_t,
                                    start=(j8 == 0), stop=(j8 == ND8 - 1),
                                    perf_mode=DR)
                    for m in range(4):
                        msl = slice(128 * m, 128 * (m + 1))
                        nc.tensor.matmul(vacc[m][:, :LOCAL_F],
                                         x_t[:, :, msl], wv8_t[j8],
                                         start=(j8 == 0), stop=(j8 == ND8 - 1),
                                         perf_mode=DR)
            else:
                for j in range(ND):
                    # t=4 x tiles land in xpoolB (always-fresh slots: a
                    # slot-wait at the DMA queue head would delay the x8
                    # loads queued behind it) and stay resident for phase
                    # B's Q/K sweep of the same group
                    if t == 4:
                        x_t = xpoolB.tile([128, TG], mm, tag="x")
                        x4_tiles.append(x_t)
                    else:
                        x_t = xpool.tile([128, TG], mm, tag="x")
                    # alternate x between the sync and scalar HWDGE queues:
                    # one queue alone can't feed the t=0 j-loop fast enough
                    (nc.sync if j % 2 == 0 else nc.scalar).dma_start(
                        out=x_t, in_=xT[t // 4][j])
                    if b0:
                        for wi, w_t in enumerate((wq_t, wk_t)):
                            for h in range(LOCAL_H):
                                hsl = slice(128 * h, 128 * (h + 1))
                                nc.tensor.matmul(acc[2 * wi + h], w_t[j][:, hsl],
                                                 x_t,
                                                 start=(j == 0), stop=(j == ND - 1))
                    for m in range(4):
                        msl = slice(128 * m, 128 * (m + 1))
                        nc.tensor.matmul(vacc[m][:, :LOCAL_F], x_t[:, msl],
                                         wv_t[j],
                                         start=(j == 0), stop=(j == ND - 1))
            vsc = (1.0 / WSCALE) if fp8g else 1.0
            if b0:
                for wi, res in ((0, qt), (1, kt)):
                    for h in range(LOCAL_H):
                        rope_evac(acc[2 * wi + h], res[h][:, tsl], csl,
                                  scalar_evac, vsc)
            for m in range(4):
                kb = 4 * t + m
                scalar_evac(v_sb[0][kb], vacc[m][:, :HD], vsc)
                vector_evac(v_sb[1][kb], vacc[m][:, HD:LOCAL_F], vsc)

    with (
        tc.tile_pool(name="psA2", bufs=4, space="PSUM") as psA2,
        tc.tile_pool(name="psscA", bufs=2, space="PSUM") as psscA,
        tc.tile_pool(name="pspvA", bufs=2, space="PSUM") as pspvA,
    ):
        PS['sc'] = psscA
        PS['pv'] = pspvA
        for t in (5, 6, 7):
            gi = FP8_GROUPS.index(t)
            vacc = [psA2.tile([128, TG], f32, tag="qk", name=f"vacc{i}")
                    for i in range(4)]
            for j8 in range(ND8):
                x_t = xpool8.tile([128, 2, TG], f8, tag="x8")
                (nc.sync if j8 % 2 == 0 else nc.scalar).dma_start(
                    out=x_t, in_=x8d[gi][j8])
                for m in range(4):
                    msl = slice(128 * m, 128 * (m + 1))
                    nc.tensor.matmul(vacc[m][:, :LOCAL_F],
                                     x_t[:, :, msl], wv8_t[j8],
                                     start=(j8 == 0), stop=(j8 == ND8 - 1),
                                     perf_mode=DR)
            for m in range(4):
                kb = 4 * t + m
                scalar_evac(v_sb[0][kb], vacc[m][:, :HD], 1.0 / WSCALE)
                vector_evac(v_sb[1][kb], vacc[m][:, HD:LOCAL_F], 1.0 / WSCALE)
            for (h, b, g) in A_UNITS[t]:
                emit_unit(h, b, g)

    # PSUM for phases B/C: 3 (Q/K sweep accs) + 3 (scores+sum) + 2 (PV)
    psum_stack = [tc.tile_pool(name="psqk", bufs=3, space="PSUM"),
                  tc.tile_pool(name="pssc", bufs=3, space="PSUM"),
                  tc.tile_pool(name="pspv", bufs=2, space="PSUM")]
    psqk, pssc, pspv = [s.__enter__() for s in psum_stack]
    PS['sc'] = pssc
    PS['pv'] = pspv

    # ---------------- phase B: b1 Q/K sweeps + ALL attention units
    # (units slotted between sweeps so the EXP load -- the ACT engine is
    # the attention bottleneck -- hides under projection matmuls; b1 unit
    # (h,1,g) becomes ready after sweep t=4+g)
    B_UNITS = {4: [(0, 0, 3)],
               5: [(1, 0, 2), (0, 1, 0)],
               6: [(0, 1, 1), (0, 1, 2)],
               7: [(0, 1, 3)]}
    for t in range(4, NT):
        # rope PSUM evacuation on DVE here: the ACT queue is deep in EXPs
        qk_sweeps(t, nc.scalar.dma_start, vector_evac,
                  xts_pre=x4_tiles if t == 4 else None)
        if t == 4:
            # gate + wo loads ride the gpsimd queue here: the gate reads
            # kt[1]'s freshly-roped t=4 columns, so the 36-tile Wo stream
            # starts ~190us in (phase B's DMA lull), not at t=0
            emit_gated_wo_loads(kt[1][:, TG * 5 - 4:TG * 5])
        for (h, b, g) in B_UNITS[t]:
            emit_unit(h, b, g)
    # AllToAll h0 fires as soon as the last h0 unit lands; the five h1
    # units held back here (~27us of PE+ACT work) cover its rendezvous
    # skew + transfer, so o_proj pass 1 starts with the data already home.
    # Biggest units first so the last unit's normalize+DMA tail is short
    # and AllToAll h1 triggers before AllToAll h0 finishes its transfer.
    nc.gpsimd.collective_compute(
        "AllToAll", mybir.AluOpType.bypass,
        replica_groups=[list(range(N_CORES))],
        ins=[inb[0].opt()], outs=[outb[0].opt()],
    )
    for (h, b, g) in [(1, 0, 3), (1, 1, 3), (1, 1, 2), (1, 1, 1),
                      (1, 1, 0)]:
        emit_unit(h, b, g)
    nc.gpsimd.collective_compute(
        "AllToAll", mybir.AluOpType.bypass,
        replica_groups=[list(range(N_CORES))],
        ins=[inb[1].opt()], outs=[outb[1].opt()],
    )

    # close attention-era PSUM pools so o_proj gets its banks, and the
    # attention-era SBUF pools so afull/wop/outp fit
    for s in reversed(psum_stack):
        s.__exit__(None, None, None)
    for s in reversed(stack_attn):
        s.__exit__(None, None, None)

    # ---------------- phase C: output projection for my 512-token slice
    with (
        tc.tile_pool(name="afull", bufs=D // 128) as afull,
        tc.tile_pool(name="outp", bufs=4) as outp,
        tc.tile_pool(name="psop", bufs=4, space="PSUM") as psop,
    ):
        af = [None] * (D // 128)

        def load_af(f):
            a_t = afull.tile([128, TG], mm, tag="af", name=f"af{f}")
            # alternate queues: the 1MB burst right after each AllToAll
            # lands in half the time split across two HWDGEs
            eng = nc.sync if (f // LOCAL_H) % 2 == 0 else nc.scalar
            eng.dma_start(out=a_t, in_=outb[f % LOCAL_H][f // LOCAL_H])
            af[f] = a_t

        for f in range(0, D // 128, LOCAL_H):  # head-0 features for pass 1
            load_af(f)
        # pass 1: head-0 feature chunks only -- these land with the first
        # AllToAll, so this entire pass overlaps the second collective.
        # Partial sums are stashed in the dead qt tiles.
        evens = [f for f in range(ND) if f % LOCAL_H == 0]
        odds = [f for f in range(ND) if f % LOCAL_H != 0]
        # emit the remaining wo loads (cycle-safe: we're past the collective
        # triggers in the gpsimd queue; each entry waits only on its pool
        # slot, which pass-1 consumption releases in load order)
        load_wo_upto(63)
        for n in range(4):
            for m in range(4):
                p = 4 * n + m
                ps = psop.tile([128, TG], f32, tag="op", name="op1")
                for i, f in enumerate(evens):
                    nc.tensor.matmul(ps, af[f][:, 128 * m:128 * (m + 1)],
                                     wo_t[(n, f)],
                                     start=(i == 0), stop=(i == len(evens) - 1))
                # DVE, not ACT: the ACT queue is still draining the held-back
                # units' EXPs when pass 1 starts
                nc.vector.tensor_copy(
                    qt[p // 8][:, TG * (p % 8):TG * (p % 8 + 1)], ps)
        # pass 2: head-1 feature chunks + the stashed partial
        for f in range(1, D // 128, LOCAL_H):
            load_af(f)
        for n in range(4):
            nsl = slice(TG * n, TG * (n + 1))
            for m in range(4):
                p = 4 * n + m
                ps = psop.tile([128, TG], f32, tag="op", name="op2")
                for i, f in enumerate(odds):
                    nc.tensor.matmul(ps, af[f][:, 128 * m:128 * (m + 1)],
                                     wo_t[(n, f)],
                                     start=(i == 0), stop=(i == len(odds) - 1))
                o_t = outp.tile([128, TG], f32, tag="o")
                nc.vector.tensor_add(
                    o_t, ps,
                    qt[p // 8][:, TG * (p % 8):TG * (p % 8 + 1)])
                # split the 4MB of out writes across both HWDGEs (ACT is
                # idle in phase C) so the final drain halves
                (nc.scalar if m % 2 == 0 else nc.sync).dma_start(
                    out=out[128 * m:128 * (m + 1), nsl], in_=o_t)
    for s in reversed(stack):
        s.__exit__(None, None, None)


# ---------------------------------------------------------------- entry point
def kernel(x, mask, Wq, Wk, Wv, Wo):
    global last_exec_time_ns
    from concourse.bass_utils import run_bass_kernel_spmd

    x = np.asarray(x, dtype=np.float32)
    mask2d = np.ascontiguousarray(np.asarray(mask, dtype=np.float32)[0, 0])
    Wq = np.asarray(Wq, dtype=np.float32)
    Wk = np.asarray(Wk, dtype=np.float32)
    Wv = np.asarray(Wv, dtype=np.float32)
    Wo = np.ascontiguousarray(np.asarray(Wo, dtype=np.float32))

    # ---- host-side prep
    import ml_dtypes
    F8 = ml_dtypes.float8_e4m3  # TRN FP8_EXP4: e4m3 with inf, max +-240

    def q8(a):
        return np.clip(a, -240.0, 240.0).astype(F8)

    cls = _classify_mask(mask2d)
    plan = _build_plan(cls)
    mask01 = None
    mask_idx = {}
    strips = []
    strip_of = {}  # content hash -> strip index (causal masks dedupe to 1)
    for (g, j), (comp, a0, nb) in sorted(plan.items()):
        if nb == 0:
            continue
        if mask01 is None:
            mask01 = np.ascontiguousarray(mask2d.T)
        q0 = 512 * g + 128 * a0
        strip = np.ones((128, 512), dtype=np.float32)
        strip[:, :128 * nb] = (mask01[128 * j:128 * (j + 1),
                                      q0:q0 + 128 * nb] == 0.0)
        hkey = strip.tobytes()
        if hkey not in strip_of:
            strips.append(strip)
            strip_of[hkey] = len(strips) - 1
        mask_idx[(g, j)] = (strip_of[hkey], nb)
    maskc = (np.stack(strips).astype(np.float16) if strips
             else np.zeros((1, 128, 512), dtype=np.float16))

    xf = np.ascontiguousarray(x.reshape(TOKS, D))
    # fp16 x tiles for groups 0 and 4 only (the rest ship as fp8):
    # xt16[a, j, p, n] = x[512*(4a) + n, 128 j + p], contiguous per tile
    xt16 = np.empty((2, D // 128, 128, TG), np.float16)
    for a, g in enumerate((0, 4)):
        blk = xf[TG * g:TG * (g + 1)]            # [512, D]
        xt16[a] = blk.T.reshape(D // 128, 128, TG)
    cosT, sinT = _rope_tables()

    # fp8 pair-tiled x for the fp8 token groups: x8[gi, j8, p, i, n] =
    # x[512g + n, 256 j8 + 128 i + p]
    x8 = np.empty((len(FP8_GROUPS), ND8, 128, 2, TG), F8)
    for gi, g in enumerate(FP8_GROUPS):
        blk = xf[TG * g:TG * (g + 1)]            # [512, D]
        t = blk.T.reshape(ND8, 2, 128, TG)       # [j8, i, p, n]
        x8[gi] = q8(t.transpose(0, 2, 1, 3))

    def wpair(Wsl):  # [D, F] -> [ND8, 128, 2, F] fp8, pre-scaled
        return q8((Wsl * WSCALE).reshape(ND8, 2, 128, -1).transpose(0, 2, 1, 3))

    key = _plan_key(plan)
    if key not in _NC_CACHE:
        _NC_CACHE[key] = _build(plan, len(strips), mask_idx)
    nc = _NC_CACHE[key]
    ones = np.ones((128, 128), dtype=np.float16)

    in_maps = []
    for c in range(N_CORES):
        fsl = slice(LOCAL_F * c, LOCAL_F * (c + 1))
        in_maps.append({
            "xT": xt16,
            "x8": x8,
            "wq": np.ascontiguousarray(Wq[:, fsl].astype(np.float16)),
            "wk": np.ascontiguousarray(Wk[:, fsl].astype(np.float16)),
            "wv": np.ascontiguousarray(Wv[:, fsl].astype(np.float16)),
            "wq8": wpair(Wq[:, fsl]),
            "wk8": wpair(Wk[:, fsl]),
            "wv8": wpair(Wv[:, fsl]),
            "wo": Wo.astype(np.float16),
            "maskc": maskc,
            "cosT": cosT,
            "sinT": sinT,
            "onesd": ones,
        })

    trace = bool(os.environ.get("KERNEL_TRACE"))
    err = None
    for attempt in range(4):
        try:
            res = run_bass_kernel_spmd(nc, in_maps,
                                       core_ids=list(range(N_CORES)),
                                       trace=trace and attempt < 2)
            break
        except ImportError:
            # tracing infra unavailable in this environment; run untraced
            trace = False
        except Exception as e:  # axon transport can be flaky; retry
            err = e
    else:
        raise err

    last_exec_time_ns = res.exec_time_ns
    kernel.last_result = res
    out_flat = np.concatenate([res.results[c]["out"] for c in range(N_CORES)],
                              axis=0)
    return out_flat.reshape(B, S, D)



# revision 73
# speedup vs baseline: 1.0213x; 1.0213x over previous
"""Trainium2 Bass kernel for HF-style causal self-attention (B=2, S=2048, D=2048,
H=16, head_dim=128), tensor-parallel over heads across 8 NeuronCores.

Sharding: core c computes heads {2c, 2c+1} for both batches (column-sharded
Wq/Wk/Wv). After per-head attention, an 8-rank AllToAll redistributes the
per-head context from head-sharding to token-sharding, and each core runs the
output projection (full Wo) for its 512-token slice. The host concatenates
the 8 token slices.

Matmul operands are fp16 (1 cycle/row on PE, FWL weight loads), EXCEPT the
Q/K/V projections of token groups 1-3 and 5-7, which run fp8-e4m3 with
perf_mode=DoubleRow: operands are pair-tiled [128, 2, n] so each matmul
contracts 256 deep in ~n cycles -- 1.77x the fp16 projection rate. Groups 0
and 4 (each batch's first 512 tokens) stay fp16: early tokens attend to few
keys, so their q/k/v quantization errors pass the softmax unattenuated,
while late tokens average ~1000 keys and fp8 noise washes out (measured
max-rel 4.2e-3 vs the 2e-2 gate; fp16 baseline was 4.4e-4). Weights are
pre-scaled x32 before e4m3 quantization (else half the N(0, 1/D) weights
land subnormal) and the 1/32 is folded into the PSUM evacuations.
Attention and o_proj stay fp16 (attention contracts only 128 so DoubleRow
can't pair; o_proj feeds the output directly and fp8 there fails the
early-token error budget). PSUM accumulation is fp32; the output is fp32.

Schedule (the point of this version): a soft pipeline. Projections run as
separate Q/K/V SWEEPS (one accumulator pair at a time, so projections +
attention PSUM pools coexist inside 8 banks), and the 32 attention units are
hand-placed into slots between sweeps so the softmax EXPs (ACT engine, the
attention bottleneck at ~110us total) hide under projection matmuls:

  A: per batch-0 group: Q/K sweeps + RoPE + V sweeps; V-only for batch-1
     groups; early batch-0 units interleave after their deps land.
  B: per batch-1 group: Q/K sweeps + RoPE, then the (b0 leftovers and) b1
     units of the matching group. The h0 AllToAll fires before the last h1
     unit; the h1 AllToAll right after it.
  C: o_proj pass 1 (head-0 features, under the h1 collective), pass 2.

Attention unit math: scores^T [k,q] chunks on PE, causally trimmed via a
runtime mask classification; exp on ACT (scale 1/sqrt(hd), no
max-subtraction -- unit-variance inputs); causal mask applied POST-exp as a
0/1 fp16 multiply; softmax denominators by DVE elementwise accumulation of
prob chunks + ONE ones-matmul per unit; normalization fused into the PSUM
evacuation (reciprocal+multiply on DVE).
"""

import math
import os

import numpy as np

# ---------------------------------------------------------------- constants
B, S, D = 2, 2048, 2048
H, HD = 16, 128
N_CORES = 8
LOCAL_H = H // N_CORES  # 2 heads per core
LOCAL_F = LOCAL_H * HD  # 256 local features
TOKS = B * S  # 4096 flat tokens (batch-major)
TG = 512  # token-group width (matmul moving dim)
NT = TOKS // TG  # 8 token groups
NB = S // 128  # 16 key blocks per batch
QB = TG // 128  # 4 query blocks per group
ROPE_BASE = 10000.0
SCALE = 1.0 / math.sqrt(HD)
SKIP_THRESH = -1e8  # mask block entirely masked if all values below this
LA = 4  # scores/exp chunks emitted ahead of PV (hides ACT latency from PE)

# fp8 (e4m3, DoubleRow) config: projections for all token groups EXCEPT the
# first 512 tokens of each batch (groups 0 and 4) run with fp8 weights+x --
# early tokens attend to few keys, so their q/k/v errors pass through the
# softmax unattenuated; late tokens average ~1000 keys and fp8 noise washes
# out. o_proj runs fp8 for each core's token sub-blocks 1-3 (sub-block 0
# holds the global early tokens on cores 0 and 4). Measured (numpy emulation
# vs fp32 reference): max-rel 5.9e-3 vs the 2e-2 gate.
FP8_GROUPS = (1, 2, 3, 5, 6, 7)
WSCALE = 32.0   # weight pre-scale before e4m3 quantization
AF8S = 8.0      # attention-out pre-scale for the fp8 o_proj path
OSCALE = WSCALE * AF8S  # o_proj PSUM carries OSCALE * value
ND8 = D // 256  # 8 paired contraction chunks for DoubleRow
# o_proj feature-pair order: pass-1 pairs (even heads), then pass-2 (odd)
O_PAIRS = [(0, 2), (4, 6), (8, 10), (12, 14), (1, 3), (5, 7), (9, 11),
           (13, 15)]

_NC_CACHE: dict = {}
last_exec_time_ns = None


# ---------------------------------------------------------------- host prep
def _rope_tables():
    inv_freq = 1.0 / (ROPE_BASE ** (np.arange(0, HD, 2, dtype=np.float64) / HD))
    t = np.arange(S, dtype=np.float64)
    freqs = np.outer(t, inv_freq)  # [S, HD/2]
    emb = np.concatenate([freqs, freqs], axis=-1)  # [S, HD]
    cos = np.cos(emb).T  # [HD, S]
    sin = np.sin(emb).T
    # rotate_half with the sign folded into a partition-aligned sin table:
    #   t2[0:64]   = s[64:128] * sinC[64:128]   (sinC[64:] = -sin[0:64])
    #   t2[64:128] = s[0:64]   * sinC[0:64]     (sinC[:64] = sin[64:128])
    # (DVE requires both SBUF inputs at the same base partition, so the
    # table rows live at the SOURCE partition of s, not the output's)
    half = HD // 2
    sinc = np.empty_like(sin)
    sinc[:half] = sin[half:]
    sinc[half:] = -sin[:half]
    return (np.ascontiguousarray(cos.astype(np.float16)),
            np.ascontiguousarray(sinc.astype(np.float16)))


def _classify_mask(mask2d):
    """Per 128x128 block of mask[q, k]: 0=all-zero, 1=needs mul, 2=fully masked."""
    nq, nk = S // 128, S // 128
    blocks = mask2d.reshape(nq, 128, nk, 128)
    mx = blocks.max(axis=(1, 3))
    mn = blocks.min(axis=(1, 3))
    cls = np.ones((nq, nk), dtype=np.int8)
    cls[(mx == 0.0) & (mn == 0.0)] = 0
    cls[mx < SKIP_THRESH] = 2
    return cls


def _build_plan(cls):
    """For each (query group g, key block j): None if skipped, else
    (comp_start_lqb, mul_start_lqb, mul_nblocks). The mul range spans the
    first to last local query block needing the 0/1 mask multiply."""
    plan = {}
    for g in range(4):
        for j in range(NB):
            lcls = [cls[4 * g + l, j] for l in range(QB)]
            if all(c == 2 for c in lcls):
                continue
            comp = min(l for l in range(QB) if lcls[l] != 2)
            muls = [l for l in range(comp, QB) if lcls[l] != 0]
            if muls:
                plan[(g, j)] = (comp, muls[0], muls[-1] - muls[0] + 1)
            else:
                plan[(g, j)] = (comp, 0, 0)
    return plan


def _plan_key(plan):
    return tuple(sorted((k, v) for k, v in plan.items()))


# ---------------------------------------------------------------- bass build
def _build(plan, n_mask_blocks, mask_idx):
    import concourse.bacc as bacc
    import concourse.mybir as mybir
    import concourse.tile as tile

    f32 = mybir.dt.float32
    mm = mybir.dt.float16
    f8 = mybir.dt.float8e4

    nc = bacc.Bacc("TRN2", target_bir_lowering=False, debug=False,
                   num_devices=N_CORES)

    # fp16 x, host-pretiled per (group in {0,4}, j): each [128, TG] tile is
    # one contiguous 128KB DRAM block (the old xT [D, TOKS] view produced
    # 1KB-strided descriptor chains that throttled the t=0 stream)
    xT = nc.dram_tensor("xT", [2, D // 128, 128, TG], mm,
                        kind="ExternalInput").ap()
    wq = nc.dram_tensor("wq", [D, LOCAL_F], mm, kind="ExternalInput").ap()
    wk = nc.dram_tensor("wk", [D, LOCAL_F], mm, kind="ExternalInput").ap()
    wv = nc.dram_tensor("wv", [D, LOCAL_F], mm, kind="ExternalInput").ap()
    # fp8 pair-tiled operands (DoubleRow): x8[gi][j8] pairs d-chunks
    # (256j8+128i+p); w*8[j8] likewise; wo8[ni][pi] pairs feature chunks
    x8d = nc.dram_tensor("x8", [len(FP8_GROUPS), ND8, 128, 2, TG], f8,
                         kind="ExternalInput").ap()
    wq8d = nc.dram_tensor("wq8", [ND8, 128, 2, LOCAL_F], f8,
                          kind="ExternalInput").ap()
    wk8d = nc.dram_tensor("wk8", [ND8, 128, 2, LOCAL_F], f8,
                          kind="ExternalInput").ap()
    wv8d = nc.dram_tensor("wv8", [ND8, 128, 2, LOCAL_F], f8,
                          kind="ExternalInput").ap()
    wo = nc.dram_tensor("wo", [D, D], mm, kind="ExternalInput").ap()
    maskc = nc.dram_tensor("maskc", [max(n_mask_blocks, 1), 128, 512], mm,
                           kind="ExternalInput").ap()
    cosT = nc.dram_tensor("cosT", [HD, S], mm, kind="ExternalInput").ap()
    sinT = nc.dram_tensor("sinT", [HD, S], mm, kind="ExternalInput").ap()
    onesd = nc.dram_tensor("onesd", [128, 128], mm, kind="ExternalInput").ap()
    out = nc.dram_tensor("out", [TG, D], f32, kind="ExternalOutput").ap()

    with tile.TileContext(nc) as tc:
        with (
            tc.tile_pool(name="const", bufs=1) as constp,
            tc.tile_pool(name="dram", bufs=1, space="DRAM") as dram,
        ):
            cos_t = constp.tile([HD, S], mm, tag="cos")
            sin_t = constp.tile([HD, S], mm, tag="sin")
            ones_t = constp.tile([128, 128], mm, tag="ones")

            _attention_body(nc, tc, tile, mybir, mm, plan, mask_idx,
                            cos_t, sin_t, ones_t,
                            xT, wq, wk, wv, wo, out, dram,
                            maskc, cosT, sinT, onesd,
                            x8d, wq8d, wk8d, wv8d)

    nc.compile()
    return nc


def _attention_body(nc, tc, tile, mybir, mm, plan, mask_idx,
                    cos_t, sin_t, ones_t,
                    xT, wq, wk, wv, wo, out, dram,
                    maskc, cosT, sinT, onesd,
                    x8d, wq8d, wk8d, wv8d):
    f32 = mybir.dt.float32
    f8 = mybir.dt.float8e4
    Exp = mybir.ActivationFunctionType.Exp
    DR = mybir.MatmulPerfMode.DoubleRow
    ND = D // 128  # 16 contraction chunks
    half = HD // 2

    inb = [dram.tile([N_CORES, HD, TG], mm, name=f"inb{i}")
           for i in range(LOCAL_H)]
    outb = [dram.tile([N_CORES, HD, TG], mm, name=f"outb{i}")
            for i in range(LOCAL_H)]

    # pools that must survive into phase C (qt doubles as the o_proj
    # pass-1 stash; wop holds the Wo tiles)
    stack = [
        tc.tile_pool(name="qkres", bufs=LOCAL_H),
        tc.tile_pool(name="wop", bufs=40),
    ]
    (qkres, wop) = [s.__enter__() for s in stack]
    # attention-era pools: closed before phase C so afull/outp fit in SBUF
    stack_attn = [
        tc.tile_pool(name="vres", bufs=LOCAL_H * 4 * NT),
        tc.tile_pool(name="wpool", bufs=3 * ND),
        tc.tile_pool(name="wpool8", bufs=3 * ND8),
        tc.tile_pool(name="xpool", bufs=16),
        tc.tile_pool(name="xpool8", bufs=10),
        tc.tile_pool(name="xpoolB", bufs=16),
        tc.tile_pool(name="ropes", bufs=3),
        tc.tile_pool(name="ropet", bufs=2),
        tc.tile_pool(name="ropeu", bufs=2),
        tc.tile_pool(name="maskp", bufs=1),
        tc.tile_pool(name="probs", bufs=10),
        tc.tile_pool(name="saccp", bufs=4),
        tc.tile_pool(name="recipp", bufs=3),
        tc.tile_pool(name="attnp", bufs=5),
    ]
    (vres, wpool, wpool8, xpool, xpool8, xpoolB, ropes, ropet1, ropet2,
     maskp, probs, saccp, recipp, attnp) = [s.__enter__() for s in stack_attn]

    # resident Q^T / K^T per local head [128, TOKS] (fp16)
    qt = [qkres.tile([HD, TOKS], mm, tag="qt", name=f"qt{i}") for i in range(LOCAL_H)]
    kt = [qkres.tile([HD, TOKS], mm, tag="kt", name=f"kt{i}") for i in range(LOCAL_H)]
    # resident V tiles [128 tok, HD] per (local head, flat token block):
    # per-head contiguous so the PV matmul's LDWEIGHTS is FWL-eligible
    v_sb = [[vres.tile([128, HD], mm, tag="v", name=f"v{h}_{i}")
             for i in range(TOKS // 128)] for h in range(LOCAL_H)]

    wq_t = [wpool.tile([128, LOCAL_F], mm, tag="w", name=f"wqt{i}") for i in range(ND)]
    wk_t = [wpool.tile([128, LOCAL_F], mm, tag="w", name=f"wkt{i}") for i in range(ND)]
    wv_t = [wpool.tile([128, LOCAL_F], mm, tag="w", name=f"wvt{i}") for i in range(ND)]
    wq8_t = [wpool8.tile([128, 2, LOCAL_F], f8, tag="w8", name=f"wq8t{i}")
             for i in range(ND8)]
    wk8_t = [wpool8.tile([128, 2, LOCAL_F], f8, tag="w8", name=f"wk8t{i}")
             for i in range(ND8)]
    wv8_t = [wpool8.tile([128, 2, LOCAL_F], f8, tag="w8", name=f"wv8t{i}")
             for i in range(ND8)]


    # Q/K weights (fp16 + fp8) ride the gpsimd queue BEFORE the barrier
    # trigger: that queue is otherwise idle until the barrier, and this
    # sheds 3MB from the sync/scalar queues during the t=0 ramp, when the
    # j-loop consumes 320KB every ~1.6us and any queue lag stalls the PE.
    # The barrier trigger moves ~15us later, still finishing long before
    # its dependents (mask strips, needed at the first unit ~190us).
    # Interleaved wq/wk order matches the j-loop's consumption order.
    for j in range(ND):
        nc.gpsimd.dma_start(out=wq_t[j], in_=wq[128 * j:128 * (j + 1), :])
        nc.gpsimd.dma_start(out=wk_t[j], in_=wk[128 * j:128 * (j + 1), :])
        nc.gpsimd.dma_start(out=wv_t[j], in_=wv[128 * j:128 * (j + 1), :])
    # rope/ones constants follow the fp16 weights on gpsimd (land ~20us,
    # first rope evac needs them ~22us) -- keeping their 1MB off the scalar
    # queue lets the t=0 x-odd tiles start immediately
    nc.gpsimd.dma_start(out=cos_t, in_=cosT)
    nc.gpsimd.dma_start(out=sin_t, in_=sinT)
    nc.gpsimd.dma_start(out=ones_t, in_=onesd)
    for j8 in range(ND8):
        nc.gpsimd.dma_start(out=wq8_t[j8], in_=wq8d[j8])
        nc.gpsimd.dma_start(out=wk8_t[j8], in_=wk8d[j8])
        nc.gpsimd.dma_start(out=wv8_t[j8], in_=wv8d[j8])

    # mask strips ride the gpsimd queue BEFORE the barrier: deduped by
    # content on the host (a causal mask has ONE distinct 128KB strip), so
    # they cost nothing at t=0 and are resident for the early attention
    # units that now run inside phase A.
    mask_tiles = {}
    mt_of_idx = {}
    for key, (idx, nb) in sorted(mask_idx.items()):
        if idx not in mt_of_idx:
            mt = maskp.tile([128, 512], mm, tag=f"mb{idx}", name=f"mb{idx}")
            nc.gpsimd.dma_start(out=mt, in_=maskc[idx])
            mt_of_idx[idx] = mt
        mask_tiles[key] = mt_of_idx[idx]

    # a tiny rendezvous at kernel start aligns the cores while phase A's
    # DMAs stream, so the real collectives later pay less skew
    barrier_in = dram.tile([N_CORES, 128, 2], mm, name="barrier_in")
    barrier_out = dram.tile([N_CORES, 128, 2], mm, name="barrier_out")
    nc.gpsimd.collective_compute(
        "AllToAll", mybir.AluOpType.bypass,
        replica_groups=[list(range(N_CORES))],
        ins=[barrier_in.opt()], outs=[barrier_out.opt()],
    )

    wo_t = {}
    # Load order = consumption order: evens (o_proj pass 1, head-0 features)
    # for all n-groups, then odds (pass 2). One shared tag, so slots released
    # by pass 1 recycle into odd-tile prefetch while pass 1 still runs.
    _wo_order = ([(n, f) for n in range(4) for f in range(0, ND, 2)]
                 + [(n, f) for n in range(4) for f in range(1, ND, 2)])

    def load_wo(k):
        n, f = _wo_order[k]
        w_t = wop.tile([128, TG], mm, tag="wo", name=f"wo{n}_{f}")
        nc.gpsimd.dma_start(out=w_t,
                            in_=wo[128 * f:128 * (f + 1),
                                   TG * n:TG * (n + 1)])
        wo_t[(n, f)] = w_t

    # The first 40 wo tiles (all of pass 1 + all of n=0 pass 2) are emitted up front
    # on the gpsimd queue but BEHIND a gate DMA whose input is written by
    # phase B's sweep t=4 RoPE -- so Wo streams during phase B's DMA lull
    # instead of competing with x/weights at t=0 or with the af/out traffic
    # in phase C. The remaining 24 are emitted inside the o_proj loop AFTER
    # the collective triggers, so the in-order gpsimd queue can't cycle
    # (they wait on pass-1 slot releases, which need AllToAll-1).
    _wo_next = [40]

    def emit_gated_wo_loads(gate_src):
        gate_dram = dram.tile([128, 4], mm, name="wo_gate")
        nc.gpsimd.dma_start(out=gate_dram, in_=gate_src)
        for k in range(40):
            load_wo(k)

    def load_wo_upto(k):
        while _wo_next[0] <= k:
            load_wo(_wo_next[0])
            _wo_next[0] += 1

    def rope_evac(ps, dst, csl, evac_eng, scale=1.0):
        # RoPE on DVE: dst = s*cos + rot_half(s)*sin_signed
        # scale un-does the fp8 weight pre-scale during the PSUM evacuation
        s_t = ropes.tile([HD, TG], mm, tag="s")
        evac_eng(s_t, ps, scale)  # fp32 PSUM -> fp16 SBUF
        t1 = ropet1.tile([HD, TG], mm, tag="t1")
        nc.vector.tensor_mul(t1, s_t, cos_t[:, csl])
        t2 = ropet2.tile([HD, TG], mm, tag="t2")
        nc.vector.tensor_mul(t2[:half], s_t[half:], sin_t[half:, csl])
        nc.vector.tensor_mul(t2[half:], s_t[:half], sin_t[:half, csl])
        nc.vector.tensor_add(dst, t1, t2)

    def scalar_evac(dst, src, scale=1.0):
        if scale == 1.0:
            nc.scalar.copy(dst, src)
        else:
            nc.scalar.mul(dst, src, scale)

    def vector_evac(dst, src, scale=1.0):
        if scale == 1.0:
            nc.vector.tensor_copy(dst, src)
        else:
            nc.vector.tensor_scalar_mul(dst, src, scale)

    def qk_sweeps(t, xdma, evac_eng, xts_pre=None):
        """Q then K projection sweep for token group t + RoPE evacuation.
        Returns nothing; qt/kt columns for group t become valid.
        xts_pre: x tiles already resident in SBUF (phase A t=4 leftovers)."""
        tsl = slice(TG * t, TG * (t + 1))
        csl = slice(TG * (t % 4), TG * (t % 4 + 1))
        fp8g = t in FP8_GROUPS
        if fp8g:
            gi = FP8_GROUPS.index(t)
            xts = []
            for wi, (w8t, res) in enumerate(((wq8_t, qt), (wk8_t, kt))):
                acc = [psqk.tile([HD, TG], f32, tag="qk", name=f"qk{h}")
                       for h in range(LOCAL_H)]
                for j8 in range(ND8):
                    if wi == 0:
                        x_t = xpool8.tile([128, 2, TG], f8, tag="x8")
                        xdma(out=x_t, in_=x8d[gi][j8])
                        xts.append(x_t)
                    for h in range(LOCAL_H):
                        hsl = slice(128 * h, 128 * (h + 1))
                        nc.tensor.matmul(acc[h], w8t[j8][:, :, hsl], xts[j8],
                                         start=(j8 == 0), stop=(j8 == ND8 - 1),
                                         perf_mode=DR)
                for h in range(LOCAL_H):
                    rope_evac(acc[h], res[h][:, tsl], csl, evac_eng,
                              1.0 / WSCALE)
            return xts
        xts = [] if xts_pre is None else xts_pre
        for wi, (w_t, wext, res) in enumerate(((wq_t, wq, qt),
                                               (wk_t, wk, kt))):
            acc = [psqk.tile([HD, TG], f32, tag="qk", name=f"qk{h}")
                   for h in range(LOCAL_H)]
            for j in range(ND):
                if wi == 0 and xts_pre is None:
                    x_t = xpoolB.tile([128, TG], mm, tag="x")
                    xdma(out=x_t, in_=xT[t // 4][j])
                    xts.append(x_t)
                for h in range(LOCAL_H):
                    hsl = slice(128 * h, 128 * (h + 1))
                    nc.tensor.matmul(acc[h], w_t[j][:, hsl], xts[j],
                                     start=(j == 0), stop=(j == ND - 1))
            for h in range(LOCAL_H):
                rope_evac(acc[h], res[h][:, tsl], csl, evac_eng)
        return xts

    PS = {}  # active PSUM pools for the attention units (phase A vs B)

    def emit_scores(h, b, g, j, pe):
        comp, a0, nb = pe
        co = 128 * comp
        qsl = slice(2048 * b + TG * g, 2048 * b + TG * (g + 1))
        ksl = slice(2048 * b + 128 * j, 2048 * b + 128 * (j + 1))
        sc = PS['sc'].tile([128, TG], f32, tag="sc", name="sc")
        nc.tensor.matmul(sc[:, co:], kt[h][:, ksl],
                         qt[h][:, qsl][:, co:],
                         start=True, stop=True)
        pt = probs.tile([128, TG], mm, tag="p", name="pt")
        nc.scalar.activation(pt[:, co:], sc[:, co:], Exp, scale=SCALE)
        if nb:
            # causal mask applied POST-exp as a 0/1 fp16 multiply (all-SBUF
            # 2-byte DVE op); raw scores are O(5) so exp never overflows
            mt = mask_tiles[(g, j)]
            q0 = 128 * a0
            nc.vector.tensor_mul(pt[:, q0:q0 + 128 * nb],
                                 pt[:, q0:q0 + 128 * nb], mt[:, :128 * nb])
        return pt

    def emit_unit(h, b, g):
        """One attention unit: all key chunks of (head h, batch b, query
        group g), pipelined LA chunks deep, ending with the denominator
        reduce + normalize + bounce-buffer write."""
        chunks = [(j, plan[(g, j)]) for j in range(NB) if (g, j) in plan]
        n = len(chunks)
        pv_ps = PS['pv'].tile([HD, TG], f32, tag="pv", name="pv")
        sa = saccp.tile([128, TG], mm, tag="sa", name="sacc")
        pts = []
        for i in range(n + LA):
            if i < n:
                j, pe = chunks[i]
                pts.append(emit_scores(h, b, g, j, pe))
            k = i - LA
            if k < 0 or k >= n:
                continue
            j, (comp, a0, nb) = chunks[k]
            pt = pts[k]
            co = 128 * comp
            first, last = k == 0, k == n - 1
            # softmax denominator partials: elementwise accumulate the prob
            # chunk on DVE (fp16 all-SBUF, 2x; gpsimd's software tensor_add
            # measured slower); partition-reduced once per unit by a single
            # ones-matmul below
            if first:
                nc.vector.tensor_copy(sa, pt)
            else:
                nc.vector.tensor_add(sa[:, co:], sa[:, co:], pt[:, co:])
            kb = 16 * b + j  # flat token block of this key chunk
            nc.tensor.matmul(pv_ps[:, co:], v_sb[h][kb], pt[:, co:],
                             start=first, stop=last)
        sum_ps = PS['sc'].tile([128, TG], f32, tag="sc", name="sum")
        nc.tensor.matmul(sum_ps, ones_t, sa, start=True, stop=True)
        rec = recipp.tile([128, TG], f32, tag="rec")
        # ~18-bit reciprocal; sums are in [1, ~5e3] so no edge cases
        nc.vector.reciprocal_approx_fast(out=rec, in_=sum_ps)
        at = attnp.tile([HD, TG], mm, tag="at")
        nc.vector.tensor_mul(at, pv_ps, rec)
        nc.sync.dma_start(out=inb[h][4 * b + g], in_=at)

    # ---------------- phase A: b0 full QKV + b1 V, interleaved j-loop
    # (long Q/K matmuls + rotating PSUM banks keep every LDWEIGHTS hidden;
    # consecutive matmuls must not accumulate into the SAME bank -- the
    # fill cannot overlap the drain and each MM pays ~40ns)
    x4_tiles = []
    # Early-unit placement: phase A's t>=5 stretch runs V-only sweeps (4
    # PSUM banks) with an idle ACT engine, while phase B's tail is
    # ACT-drain-bound (every unit carries ~1.4x more exp time than PE
    # time). Moving the four smallest b0 units here widens the exp window
    # by ~40us and pulls the whole AllToAll-2 trigger chain earlier.
    A_UNITS = {5: [(0, 0, 0), (1, 0, 0)], 6: [(0, 0, 1)], 7: [(1, 0, 1)]}
    with tc.tile_pool(name="psA", bufs=8, space="PSUM") as psA:
        for t in range(5):
            b0 = t < 4
            fp8g = t in FP8_GROUPS
            tsl = slice(TG * t, TG * (t + 1))
            csl = slice(TG * (t % 4), TG * (t % 4 + 1))
            if b0:
                acc = [psA.tile([HD, TG], f32, tag="qk", name=f"acc{i}")
                       for i in range(2 * LOCAL_H)]
            vacc = [psA.tile([128, TG], f32, tag="qk", name=f"vacc{i}")
                    for i in range(4)]
            if fp8g:
                # DoubleRow fp8: 8 paired 256-deep contraction chunks
                gi = FP8_GROUPS.index(t)
                for j8 in range(ND8):
                    x_t = xpool8.tile([128, 2, TG], f8, tag="x8")
                    (nc.sync if j8 % 2 == 0 else nc.scalar).dma_start(
                        out=x_t, in_=x8d[gi][j8])
                    if b0:
                        for wi, w8t in enumerate((wq8_t, wk8_t)):
                            for h in range(LOCAL_H):
                                hsl = slice(128 * h, 128 * (h + 1))
                                nc.tensor.matmul(
                                    acc[2 * wi + h], w8t[j8][:, :, hsl], x_t,
                                    start=(j8 == 0), stop=(j8 == ND8 - 1),
                                    perf_mode=DR)
                    for m in range(4):
                        msl = slice(128 * m, 128 * (m + 1))
                        nc.tensor.matmul(vacc[m][:, :LOCAL_F],
                                         x_t[:, :, msl], wv8_t[j8],
                                         start=(j8 == 0), stop=(j8 == ND8 - 1),
                                         perf_mode=DR)
            else:
                for j in range(ND):
                    # t=4 x tiles land in xpoolB (always-fresh slots: a
                    # slot-wait at the DMA queue head would delay the x8
                    # loads queued behind it) and stay resident for phase
                    # B's Q/K sweep of the same group
                    if t == 4:
                        x_t = xpoolB.tile([128, TG], mm, tag="x")
                        x4_tiles.append(x_t)
                    else:
                        x_t = xpool.tile([128, TG], mm, tag="x")
                    # alternate x between the sync and scalar HWDGE queues:
                    # one queue alone can't feed the t=0 j-loop fast enough
                    (nc.sync if j % 2 == 0 else nc.scalar).dma_start(
                        out=x_t, in_=xT[t // 4][j])
                    if b0:
                        for wi, w_t in enumerate((wq_t, wk_t)):
                            for h in range(LOCAL_H):
                                hsl = slice(128 * h, 128 * (h + 1))
                                nc.tensor.matmul(acc[2 * wi + h], w_t[j][:, hsl],
                                                 x_t,
                                                 start=(j == 0), stop=(j == ND - 1))
                    for m in range(4):
                        msl = slice(128 * m, 128 * (m + 1))
                        nc.tensor.matmul(vacc[m][:, :LOCAL_F], x_t[:, msl],
                                         wv_t[j],
                                         start=(j == 0), stop=(j == ND - 1))
            vsc = (1.0 / WSCALE) if fp8g else 1.0
            if b0:
                for wi, res in ((0, qt), (1, kt)):
                    for h in range(LOCAL_H):
                        rope_evac(acc[2 * wi + h], res[h][:, tsl], csl,
                                  scalar_evac, vsc)
            for m in range(4):
                kb = 4 * t + m
                scalar_evac(v_sb[0][kb], vacc[m][:, :HD], vsc)
                vector_evac(v_sb[1][kb], vacc[m][:, HD:LOCAL_F], vsc)

    with (
        tc.tile_pool(name="psA2", bufs=4, space="PSUM") as psA2,
        tc.tile_pool(name="psscA", bufs=2, space="PSUM") as psscA,
        tc.tile_pool(name="pspvA", bufs=2, space="PSUM") as pspvA,
    ):
        PS['sc'] = psscA
        PS['pv'] = pspvA
        for t in (5, 6, 7):
            gi = FP8_GROUPS.index(t)
            vacc = [psA2.tile([128, TG], f32, tag="qk", name=f"vacc{i}")
                    for i in range(4)]
            for j8 in range(ND8):
                x_t = xpool8.tile([128, 2, TG], f8, tag="x8")
                (nc.sync if j8 % 2 == 0 else nc.scalar).dma_start(
                    out=x_t, in_=x8d[gi][j8])
                for m in range(4):
                    msl = slice(128 * m, 128 * (m + 1))
                    nc.tensor.matmul(vacc[m][:, :LOCAL_F],
                                     x_t[:, :, msl], wv8_t[j8],
                                     start=(j8 == 0), stop=(j8 == ND8 - 1),
                                     perf_mode=DR)
            for m in range(4):
                kb = 4 * t + m
                scalar_evac(v_sb[0][kb], vacc[m][:, :HD], 1.0 / WSCALE)
                vector_evac(v_sb[1][kb], vacc[m][:, HD:LOCAL_F], 1.0 / WSCALE)
            for (h, b, g) in A_UNITS[t]:
                emit_unit(h, b, g)

    # PSUM for phases B/C: 3 (Q/K sweep accs) + 3 (scores+sum) + 2 (PV)
    psum_stack = [tc.tile_pool(name="psqk", bufs=3, space="PSUM"),
                  tc.tile_pool(name="pssc", bufs=3, space="PSUM"),
                  tc.tile_pool(name="pspv", bufs=2, space="PSUM")]
    psqk, pssc, pspv = [s.__enter__() for s in psum_stack]
    PS['sc'] = pssc
    PS['pv'] = pspv

    # ---------------- phase B: b1 Q/K sweeps + ALL attention units
    # (units slotted between sweeps so the EXP load -- the ACT engine is
    # the attention bottleneck -- hides under projection matmuls; b1 unit
    # (h,1,g) becomes ready after sweep t=4+g)
    B_UNITS = {4: [(0, 0, 2), (0, 0, 3)],
               5: [(1, 0, 2), (0, 1, 0)],
               6: [(0, 1, 1), (0, 1, 2)],
               7: [(0, 1, 3)]}
    for t in range(4, NT):
        # rope PSUM evacuation on DVE here: the ACT queue is deep in EXPs
        qk_sweeps(t, nc.scalar.dma_start, vector_evac,
                  xts_pre=x4_tiles if t == 4 else None)
        if t == 4:
            # gate + wo loads ride the gpsimd queue here: the gate reads
            # kt[1]'s freshly-roped t=4 columns, so the 36-tile Wo stream
            # starts ~190us in (phase B's DMA lull), not at t=0
            emit_gated_wo_loads(kt[1][:, TG * 5 - 4:TG * 5])
        for (h, b, g) in B_UNITS[t]:
            emit_unit(h, b, g)
    # AllToAll h0 fires as soon as the last h0 unit lands; the five h1
    # units held back here (~27us of PE+ACT work) cover its rendezvous
    # skew + transfer, so o_proj pass 1 starts with the data already home.
    # Biggest units first so the last unit's normalize+DMA tail is short
    # and AllToAll h1 triggers before AllToAll h0 finishes its transfer.
    nc.gpsimd.collective_compute(
        "AllToAll", mybir.AluOpType.bypass,
        replica_groups=[list(range(N_CORES))],
        ins=[inb[0].opt()], outs=[outb[0].opt()],
    )
    for (h, b, g) in [(1, 0, 3), (1, 1, 3), (1, 1, 2), (1, 1, 1),
                      (1, 1, 0)]:
        emit_unit(h, b, g)
    nc.gpsimd.collective_compute(
        "AllToAll", mybir.AluOpType.bypass,
        replica_groups=[list(range(N_CORES))],
        ins=[inb[1].opt()], outs=[outb[1].opt()],
    )

    # close attention-era PSUM pools so o_proj gets its banks, and the
    # attention-era SBUF pools so afull/wop/outp fit
    for s in reversed(psum_stack):
        s.__exit__(None, None, None)
    for s in reversed(stack_attn):
        s.__exit__(None, None, None)

    # ---------------- phase C: output projection for my 512-token slice
    with (
        tc.tile_pool(name="afull", bufs=D // 128) as afull,
        tc.tile_pool(name="outp", bufs=4) as outp,
        tc.tile_pool(name="psop", bufs=4, space="PSUM") as psop,
    ):
        af = [None] * (D // 128)

        def load_af(f):
            a_t = afull.tile([128, TG], mm, tag="af", name=f"af{f}")
            # alternate queues: the 1MB burst right after each AllToAll
            # lands in half the time split across two HWDGEs
            eng = nc.sync if (f // LOCAL_H) % 2 == 0 else nc.scalar
            eng.dma_start(out=a_t, in_=outb[f % LOCAL_H][f // LOCAL_H])
            af[f] = a_t

        for f in range(0, D // 128, LOCAL_H):  # head-0 features for pass 1
            load_af(f)
        # pass 1: head-0 feature chunks only -- these land with the first
        # AllToAll, so this entire pass overlaps the second collective.
        # Partial sums are stashed in the dead qt tiles.
        evens = [f for f in range(ND) if f % LOCAL_H == 0]
        odds = [f for f in range(ND) if f % LOCAL_H != 0]
        # emit the remaining wo loads (cycle-safe: we're past the collective
        # triggers in the gpsimd queue; each entry waits only on its pool
        # slot, which pass-1 consumption releases in load order)
        load_wo_upto(63)
        for n in range(4):
            for m in range(4):
                p = 4 * n + m
                ps = psop.tile([128, TG], f32, tag="op", name="op1")
                for i, f in enumerate(evens):
                    nc.tensor.matmul(ps, af[f][:, 128 * m:128 * (m + 1)],
                                     wo_t[(n, f)],
                                     start=(i == 0), stop=(i == len(evens) - 1))
                # DVE, not ACT: the ACT queue is still draining the held-back
                # units' EXPs when pass 1 starts
                nc.vector.tensor_copy(
                    qt[p // 8][:, TG * (p % 8):TG * (p % 8 + 1)], ps)
        # pass 2: head-1 feature chunks + the stashed partial
        for f in range(1, D // 128, LOCAL_H):
            load_af(f)
        for n in range(4):
            nsl = slice(TG * n, TG * (n + 1))
            for m in range(4):
                p = 4 * n + m
                ps = psop.tile([128, TG], f32, tag="op", name="op2")
                for i, f in enumerate(odds):
                    nc.tensor.matmul(ps, af[f][:, 128 * m:128 * (m + 1)],
                                     wo_t[(n, f)],
                                     start=(i == 0), stop=(i == len(odds) - 1))
                o_t = outp.tile([128, TG], f32, tag="o")
                nc.vector.tensor_add(
                    o_t, ps,
                    qt[p // 8][:, TG * (p % 8):TG * (p % 8 + 1)])
                # split the 4MB of out writes across both HWDGEs (ACT is
                # idle in phase C) so the final drain halves
                (nc.scalar if m % 2 == 0 else nc.sync).dma_start(
                    out=out[128 * m:128 * (m + 1), nsl], in_=o_t)
    for s in reversed(stack):
        s.__exit__(None, None, None)


# ---------------------------------------------------------------- entry point
def kernel(x, mask, Wq, Wk, Wv, Wo):
    global last_exec_time_ns
    from concourse.bass_utils import run_bass_kernel_spmd

    x = np.asarray(x, dtype=np.float32)
    mask2d = np.ascontiguousarray(np.asarray(mask, dtype=np.float32)[0, 0])
    Wq = np.asarray(Wq, dtype=np.float32)
    Wk = np.asarray(Wk, dtype=np.float32)
    Wv = np.asarray(Wv, dtype=np.float32)
    Wo = np.ascontiguousarray(np.asarray(Wo, dtype=np.float32))

    # ---- host-side prep
    import ml_dtypes
    F8 = ml_dtypes.float8_e4m3  # TRN FP8_EXP4: e4m3 with inf, max +-240

    def q8(a):
        return np.clip(a, -240.0, 240.0).astype(F8)

    cls = _classify_mask(mask2d)
    plan = _build_plan(cls)
    mask01 = None
    mask_idx = {}
    strips = []
    strip_of = {}  # content hash -> strip index (causal masks dedupe to 1)
    for (g, j), (comp, a0, nb) in sorted(plan.items()):
        if nb == 0:
            continue
        if mask01 is None:
            mask01 = np.ascontiguousarray(mask2d.T)
        q0 = 512 * g + 128 * a0
        strip = np.ones((128, 512), dtype=np.float32)
        strip[:, :128 * nb] = (mask01[128 * j:128 * (j + 1),
                                      q0:q0 + 128 * nb] == 0.0)
        hkey = strip.tobytes()
        if hkey not in strip_of:
            strips.append(strip)
            strip_of[hkey] = len(strips) - 1
        mask_idx[(g, j)] = (strip_of[hkey], nb)
    maskc = (np.stack(strips).astype(np.float16) if strips
             else np.zeros((1, 128, 512), dtype=np.float16))

    xf = np.ascontiguousarray(x.reshape(TOKS, D))
    # fp16 x tiles for groups 0 and 4 only (the rest ship as fp8):
    # xt16[a, j, p, n] = x[512*(4a) + n, 128 j + p], contiguous per tile
    xt16 = np.empty((2, D // 128, 128, TG), np.float16)
    for a, g in enumerate((0, 4)):
        blk = xf[TG * g:TG * (g + 1)]            # [512, D]
        xt16[a] = blk.T.reshape(D // 128, 128, TG)
    cosT, sinT = _rope_tables()

    # fp8 pair-tiled x for the fp8 token groups: x8[gi, j8, p, i, n] =
    # x[512g + n, 256 j8 + 128 i + p]
    x8 = np.empty((len(FP8_GROUPS), ND8, 128, 2, TG), F8)
    for gi, g in enumerate(FP8_GROUPS):
        blk = xf[TG * g:TG * (g + 1)]            # [512, D]
        t = blk.T.reshape(ND8, 2, 128, TG)       # [j8, i, p, n]
        x8[gi] = q8(t.transpose(0, 2, 1, 3))

    def wpair(Wsl):  # [D, F] -> [ND8, 128, 2, F] fp8, pre-scaled
        return q8((Wsl * WSCALE).reshape(ND8, 2, 128, -1).transpose(0, 2, 1, 3))

    key = _plan_key(plan)
    if key not in _NC_CACHE:
        _NC_CACHE[key] = _build(plan, len(strips), mask_idx)
    nc = _NC_CACHE[key]
    ones = np.ones((128, 128), dtype=np.float16)

    in_maps = []
    for c in range(N_CORES):
        fsl = slice(LOCAL_F * c, LOCAL_F * (c + 1))
        in_maps.append({
            "xT": xt16,
            "x8": x8,
            "wq": np.ascontiguousarray(Wq[:, fsl].astype(np.float16)),
            "wk": np.ascontiguousarray(Wk[:, fsl].astype(np.float16)),
            "wv": np.ascontiguousarray(Wv[:, fsl].astype(np.float16)),
            "wq8": wpair(Wq[:, fsl]),
            "wk8": wpair(Wk[:, fsl]),
            "wv8": wpair(Wv[:, fsl]),
            "wo": Wo.astype(np.float16),
            "maskc": maskc,
            "cosT": cosT,
            "sinT": sinT,
            "onesd": ones,
        })

    trace = bool(os.environ.get("KERNEL_TRACE"))
    err = None
    for attempt in range(4):
        try:
            res = run_bass_kernel_spmd(nc, in_maps,
                                       core_ids=list(range(N_CORES)),
                                       trace=trace and attempt < 2)
            break
        except ImportError:
            # tracing infra unavailable in this environment; run untraced
            trace = False
        except Exception as e:  # axon transport can be flaky; retry
            err = e
    else:
        raise err

    last_exec_time_ns = res.exec_time_ns
    kernel.last_result = res
    out_flat = np.concatenate([res.results[c]["out"] for c in range(N_CORES)],
                              axis=0)
    return out_flat.reshape(B, S, D)

